# revision 13
# baseline (speedup 1.0000x reference)
import sys
if '/opt/trn_rl_repo' not in sys.path:
    sys.path.insert(0, '/opt/trn_rl_repo')
import numpy as np
import ml_dtypes

import concourse.bass as bass
import concourse.tile as tile
from concourse import mybir, bacc
from concourse.bass_utils import run_bass_kernel_spmd

dt = mybir.dt
AF = mybir.ActivationFunctionType
ALU = mybir.AluOpType
BF = ml_dtypes.bfloat16

N_CORES = 8
B_PER = 2

# ---------------- host-side quantum constants (mirrors reference math) -------
_X = np.array([[0., 1.], [1., 0.]])

def _embed_np(U, q, n=4):
    out = None
    for i in range(n):
        m = U if i == q else np.eye(2)
        out = m if out is None else np.kron(out, m)
    return out

def _cnot_np(c, t):
    return _embed_np(np.diag([1., 0.]), c) + _embed_np(np.diag([0., 1.]), c) @ _embed_np(_X, t)

_RING = (_cnot_np(3, 0) @ _cnot_np(2, 3) @ _cnot_np(1, 2) @ _cnot_np(0, 1)).astype(np.complex64)
_SIGNS = np.array([[1 - 2 * ((i >> (3 - w)) & 1) for w in range(4)] for i in range(16)], np.float32)
_H2 = np.array([[1., 1.], [1., -1.]]) / np.sqrt(2.)
_CNOT01 = np.array([[1., 0, 0, 0], [0, 1, 0, 0], [0, 0, 0, 1], [0, 0, 1, 0]])
_RY = np.array([[np.cos(np.pi / 8), -np.sin(np.pi / 8)], [np.sin(np.pi / 8), np.cos(np.pi / 8)]])
_UPOOL = (np.kron(_RY, np.eye(2)) @ _CNOT01 @ np.kron(_H2, np.eye(2))).astype(np.float32)


def _rot2_np(p):
    phi, th, om = [np.float32(v) for v in p]
    c, s = np.cos(th / 2), np.sin(th / 2)
    ep = np.exp(-0.5j * (phi + om)).astype(np.complex64)
    em = np.exp(-0.5j * (phi - om)).astype(np.complex64)
    return np.array([[ep * c, -np.conj(em) * s], [em * s, np.conj(ep) * c]], np.complex64)


def _qconv_unitary_np(qp):
    U = np.eye(16, dtype=np.complex64)
    for layer in range(2):
        for q in range(4):
            U = _embed_np(_rot2_np(qp[layer, q]), q).astype(np.complex64) @ U
        U = _RING @ U
    return U


def _fold_bn(w, b, bn):
    s = (bn['g'] / np.sqrt(bn['v'] + 1e-5)).astype(np.float32)
    bias = ((b - bn['m']) * s + bn['b']).astype(np.float32)
    return (w * s[:, None, None, None]).astype(np.float32), bias


def _conv_stationary(w, n_k):
    co, ci = w.shape[0], w.shape[1]
    out = np.zeros((3, n_k, 32), np.float32)
    for dx in range(3):
        for dy in range(3):
            for c in range(ci):
                out[dx, dy * ci + c, :co] = w[:, c, dy, dx]
    return out


def make_consts(params):
    g = lambda v: np.asarray(v, np.float32)
    C = {}
    w1a, b1a = _fold_bn(g(params['c1a_w']), g(params['c1a_b']), {k: g(v) for k, v in params['bn1a'].items()})
    w1b, b1b = _fold_bn(g(params['c1b_w']), g(params['c1b_b']), {k: g(v) for k, v in params['bn1b'].items()})
    w2a, b2a = _fold_bn(g(params['c2a_w']), g(params['c2a_b']), {k: g(v) for k, v in params['bn2a'].items()})
    w2b, b2b = _fold_bn(g(params['c2b_w']), g(params['c2b_b']), {k: g(v) for k, v in params['bn2b'].items()})
    w2b, b2b = w2b[:8], b2b[:8]   # only channels 0..7 feed the quantum branch

    s1a = np.zeros((9, 32), np.float32)
    for dy in range(3):
        for dx in range(3):
            s1a[dy * 3 + dx, :16] = w1a[:, 0, dy, dx]
    C['w1a'] = s1a
    C['w1b'] = _conv_stationary(w1b, 48)
    C['w2a'] = _conv_stationary(w2a, 48)
    C['w2b'] = _conv_stationary(w2b, 96)

    def bias_tile(bias):
        t = np.zeros((128, 1), np.float32)
        for j in range(4):
            t[32 * j:32 * j + len(bias), 0] = bias
        return t
    C['b1a'], C['b1b'] = bias_tile(b1a), bias_tile(b1b)
    C['b2a'], C['b2b'] = bias_tile(b2a), bias_tile(b2b)

    U = _qconv_unitary_np(g(params['qparams']))
    Ur, Ui = np.ascontiguousarray(U.real[:, :4]), np.ascontiguousarray(U.imag[:, :4])
    Aw = np.zeros((4, 4, 4), np.float32)
    for w in range(4):
        Aw[w] = (Ur * _SIGNS[:, w:w + 1]).T @ Ur + (Ui * _SIGNS[:, w:w + 1]).T @ Ui
    Q = (_UPOOL.T @ np.diag([1., 1., -1., -1.]).astype(np.float32) @ _UPOOL)

    pairsB = [(0, 1), (1, 2), (2, 3), (0, 3)]
    pairsC = [(0, 2), (1, 3)]

    def quad_mats(A_list, n_w):
        # rows (32m+s) / (32k+s); cols (32w+s) if n_w==4 else (s)
        ncol = 128 if n_w == 4 else 32
        Wsq = np.zeros((128, ncol), np.float32)
        WB = np.zeros((128, ncol), np.float32)
        WC = np.zeros((128, ncol), np.float32)
        for w in range(n_w):
            A = A_list[w]
            for s in range(32):
                col = 32 * w + s if n_w == 4 else s
                for m in range(4):
                    Wsq[32 * m + s, col] = A[m, m]
                for k, (m, m2) in enumerate(pairsB):
                    WB[32 * k + s, col] = 2 * A[m, m2]
                for k, (m, m2) in enumerate(pairsC):
                    WC[32 * k + s, col] = 2 * A[m, m2]
        return Wsq, WB, WC

    C['qWsq'], C['qWB'], C['qWC'] = quad_mats([Aw[0], Aw[1], Aw[2], Aw[3]], 4)
    C['pWsq'], C['pWB'], C['pWC'] = quad_mats([Q], 1)
    Wn2 = np.zeros((128, 32), np.float32)
    for s in range(32):
        for m in range(4):
            Wn2[32 * m + s, s] = 1.0
    C['qWn2'] = Wn2

    at = params['attn']
    in_w, in_b = g(at['in_w']), g(at['in_b'])
    C['wqT'] = np.ascontiguousarray(in_w[0:128].T)
    C['wkT'] = np.ascontiguousarray(in_w[128:256].T)
    C['wvT'] = np.ascontiguousarray(in_w[256:384].T)
    C['bq'] = in_b[0:128].reshape(128, 1).copy()
    C['bk'] = in_b[128:256].reshape(128, 1).copy()
    C['bv'] = in_b[256:384].reshape(128, 1).copy()
    C['woT'] = np.ascontiguousarray(g(at['out_w']).T)
    C['bo'] = g(at['out_b']).reshape(128, 1).copy()
    C['ident'] = np.eye(128, dtype=np.float32)
    C['ones64'] = np.full((64, 1), 1.0 / 64.0, np.float32)

    cl = params['cls']
    fc1w, fc1b = g(cl['fc1_w']), g(cl['fc1_b'])
    fc2w, fc2b = g(cl['fc2_w']), g(cl['fc2_b'])
    fc3w, fc3b = g(cl['fc3_w']), g(cl['fc3_b'])
    bnf = {k: g(v) for k, v in cl['bnf'].items()}
    sb_ = bnf['g'] / np.sqrt(bnf['v'] + 1e-5)
    bb_ = bnf['b'] - bnf['m'] * sb_
    fc2w_eff = fc2w * sb_[None, :]
    fc2b_eff = fc2b + fc2w @ bb_
    C['fc1aT'] = np.ascontiguousarray(fc1w[0:128].T)
    C['fc1bT'] = np.ascontiguousarray(fc1w[128:256].T)
    C['bfc1a'] = fc1b[0:128].reshape(128, 1).copy()
    C['bfc1b'] = fc1b[128:256].reshape(128, 1).copy()
    C['fc2aT'] = np.ascontiguousarray(fc2w_eff[:, 0:128].T)
    C['fc2bT'] = np.ascontiguousarray(fc2w_eff[:, 128:256].T)
    C['bfc2'] = fc2b_eff.reshape(128, 1).copy()
    fc3T = np.zeros((128, 32), np.float32)
    fc3T[:, :10] = fc3w.T
    C['fc3T'] = fc3T
    b3 = np.zeros((32, 1), np.float32)
    b3[:10, 0] = fc3b
    C['bfc3'] = b3
    return C


CONST_SPECS = [
    ('w1a', [9, 32], dt.bfloat16), ('w1b', [3, 48, 32], dt.bfloat16),
    ('w2a', [3, 48, 32], dt.bfloat16), ('w2b', [3, 96, 32], dt.bfloat16),
    ('b1a', [128, 1], dt.float32), ('b1b', [128, 1], dt.float32),
    ('b2a', [128, 1], dt.float32), ('b2b', [128, 1], dt.float32),
    ('qWsq', [128, 128], dt.float32), ('qWB', [128, 128], dt.float32), ('qWC', [128, 128], dt.float32),
    ('qWn2', [128, 32], dt.float32), ('pWsq', [128, 32], dt.float32), ('pWB', [128, 32], dt.float32), ('pWC', [128, 32], dt.float32),
    ('wqT', [128, 128], dt.float32), ('wkT', [128, 128], dt.float32), ('wvT', [128, 128], dt.float32),
    ('bq', [128, 1], dt.float32), ('bk', [128, 1], dt.float32), ('bv', [128, 1], dt.float32),
    ('woT', [128, 128], dt.float32), ('bo', [128, 1], dt.float32),
    ('ident', [128, 128], dt.float32), ('ones64', [64, 1], dt.float32),
    ('fc1aT', [128, 128], dt.float32), ('fc1bT', [128, 128], dt.float32),
    ('bfc1a', [128, 1], dt.float32), ('bfc1b', [128, 1], dt.float32),
    ('fc2aT', [128, 128], dt.float32), ('fc2bT', [128, 128], dt.float32),
    ('bfc2', [128, 1], dt.float32),
    ('fc3T', [128, 32], dt.float32), ('bfc3', [32, 1], dt.float32),
]

_PROGRAM = None


def build_program():
    nc = bacc.Bacc(None, target_bir_lowering=False)
    x_in = nc.dram_tensor("x", [B_PER, 256, 256], dt.float32, kind="ExternalInput")
    out_t = nc.dram_tensor("out", [B_PER, 10], dt.float32, kind="ExternalOutput")
    cin = {}
    for name, shape, d in CONST_SPECS:
        cin[name] = nc.dram_tensor(name, shape, d, kind="ExternalInput")
    with tile.TileContext(nc) as tc:
        _emit(nc, tc, x_in, out_t, cin)
    nc.compile()
    return nc


def _emit(nc, tc, x_in, out_t, cin):
    import contextlib
    ctx = contextlib.ExitStack()
    with ctx:
        cpool = ctx.enter_context(tc.tile_pool(name="consts", bufs=1))
        big = ctx.enter_context(tc.tile_pool(name="big", bufs=1))
        work = ctx.enter_context(tc.tile_pool(name="work", bufs=2))
        ps = ctx.enter_context(tc.tile_pool(name="ps", bufs=6, space="PSUM"))
        ps2 = ctx.enter_context(tc.tile_pool(name="ps2", bufs=2, space="PSUM"))

        ct = {}
        for name, shape, d in CONST_SPECS:
            if len(shape) == 3:
                t = cpool.tile([shape[1], shape[0] * shape[2]], d, name="c_" + name)
                nc.sync.dma_start(t[:].rearrange("k (a m) -> k a m", a=shape[0]),
                                  cin[name][:].rearrange("a k m -> k a m"))
            else:
                t = cpool.tile(shape, d, name="c_" + name)
                nc.sync.dma_start(t[:], cin[name][:])
            ct[name] = t

        im9 = big.tile([9, 64 * 258], dt.bfloat16, name="im9")
        s1 = [big.tile([128, 8 * 512], dt.bfloat16, name=f"s1_{i}") for i in range(3)]
        rt = big.tile([96, 128 * 130], dt.bfloat16, name="rt")      # shared rt1b/rt2a/rt2b
        s2a = big.tile([128, 32 * 128], dt.bfloat16, name="s2a")
        s2b = big.tile([128, 8 * 512], dt.bfloat16, name="s2b")
        sx = [big.tile([128, 8 * 128], dt.bfloat16, name=f"sx_{b}") for b in range(B_PER)]

        nc.vector.memset(im9[:], 0.0)   # one-time: zero pads (interior rewritten per stripe)

        for b in range(B_PER):
            rt1b = rt[0:48, 0:64 * 258]
            # zero rt1b pad columns (chi=0,257) once per image
            rtv = rt1b.rearrange("p (r c) -> p r c", c=258)
            nc.vector.memset(rtv[:, :, 0:1], 0.0)
            nc.vector.memset(rtv[:, :, 257:258], 0.0)

            for stripe in range(4):
                y0 = 64 * stripe
                # ---- conv1a im2col build for this stripe (from HBM) ----
                if stripe == 0:
                    nc.vector.memset(im9[0:9, 0:258], 0.0)         # rho=0 rows of d=0 taps
                if stripe == 3:
                    nc.vector.memset(im9[0:9, 63 * 258:64 * 258], 0.0)
                for d_ in range(3):
                    for e in range(3):
                        t_idx = 3 * d_ + e
                        r_lo = max(0, 1 - d_ - y0)
                        r_hi = min(64, 257 - d_ - y0)
                        chi0 = max(0, 2 - e)
                        chi1 = min(258, 258 - e)
                        ncols = chi1 - chi0
                        src = x_in[b, y0 + r_lo + d_ - 1: y0 + r_hi + d_ - 1,
                                   chi0 + e - 2: chi0 + e - 2 + ncols]
                        dstv = im9[t_idx:t_idx + 1, :].rearrange("p (r c) -> p r c", c=258)
                        nc.gpsimd.dma_start(dstv[:, r_lo:r_hi, chi0:chi0 + ncols],
                                            src.rearrange("r c -> () r c"))
                # ---- conv1a matmuls ----
                s1c = s1[stripe % 3]
                im9v = im9[0:9, :].rearrange("p (rr c) -> p rr c", c=258)
                for r in range(8):
                    acc = ps.tile([128, 512], dt.float32, name="ps1a", tag="pp")
                    for j in range(4):
                        y = 8 * r + 2 * j
                        rhs = im9v[:, y:y + 2, 1:257]
                        nc.tensor.matmul(acc[32 * j:32 * j + 32, :], ct['w1a'][:], rhs,
                                         start=True, stop=True, tile_position=(0, 32 * j))
                    nc.scalar.activation(s1c[:, 512 * r:512 * (r + 1)], acc[:],
                                         AF.Relu, bias=ct['b1a'][:], scale=1.0)
                if stripe >= 1:
                    _conv1b_stripe(nc, ps, work, ct, s1, stripe - 1, rt1b, s2a)
            _conv1b_stripe(nc, ps, work, ct, s1, 3, rt1b, s2a)

            # ---------------- conv2a (full 128x128 image) -------------------
            rt2a = rt[0:48, 0:128 * 130]
            rt2av = rt2a.rearrange("p (r c) -> p r c", c=130)
            nc.vector.memset(rt2av[:, :, 0:1], 0.0)
            nc.vector.memset(rt2av[:, :, 129:130], 0.0)
            nc.vector.memset(rt2av[0:16, 0, :], 0.0)       # d=0, rho=0  (row -1)
            nc.vector.memset(rt2av[32:48, 127, :], 0.0)    # d=2, rho=127 (row 128)
            s2a_v = s2a[:].rearrange("p (sr rr x) -> p sr rr x", sr=4, rr=8)
            for d_ in range(3):
                for j in range(4):
                    for si in range(4):
                        rho0 = 32 * si + j - (d_ - 1)
                        rr_lo = 0 if rho0 >= 0 else 1
                        rr_hi = 7 if rho0 + 28 >= 128 else 8
                        if rr_lo >= rr_hi:
                            continue
                        dsl = rt2av[16 * d_:16 * d_ + 16,
                                    rho0 + 4 * rr_lo: rho0 + 4 * (rr_hi - 1) + 1: 4, 1:129]
                        ssl = s2a_v[32 * j:32 * j + 16, si, rr_lo:rr_hi, :]
                        nc.gpsimd.dma_start(dsl, ssl)
            _conv_mid(nc, ps, ct, rt2a, s2b, 'w2a', 'b2a', 48, 130)

            # ---------------- conv2b (full image) ---------------------------
            rt2b = rt[0:96, 0:128 * 130]
            rt2bv = rt2b.rearrange("p (r c) -> p r c", c=130)
            nc.vector.memset(rt2bv[:, :, 0:1], 0.0)
            nc.vector.memset(rt2bv[:, :, 129:130], 0.0)
            nc.vector.memset(rt2bv[0:32, 0, :], 0.0)
            nc.vector.memset(rt2bv[64:96, 127, :], 0.0)
            s2b_v = s2b[:].rearrange("p (rr s x) -> p rr s x", rr=8, s=4)
            for d_ in range(3):
                for j in range(4):
                    for s_ in range(4):
                        rho0 = 4 * j + s_ - (d_ - 1)
                        rr_lo = 0 if rho0 >= 0 else 1
                        rr_hi = 7 if rho0 + 112 >= 128 else 8
                        if rr_lo >= rr_hi:
                            continue
                        dsl = rt2bv[32 * d_:32 * d_ + 32,
                                    rho0 + 16 * rr_lo: rho0 + 16 * (rr_hi - 1) + 1: 16, 1:129]
                        ssl = s2b_v[32 * j:32 * j + 32, rr_lo:rr_hi, s_, :]
                        nc.gpsimd.dma_start(dsl, ssl)
            _conv2b(nc, ps, work, ct, rt2b, sx[b])

        # ================= quantum branch =================================
        qin = big.tile([128, 512], dt.float32, name="qin")
        for b in range(B_PER):
            sxv = sx[b][:].rearrange("p (r i pw two) -> p r i pw two", r=8, i=2, two=2)
            qinv = qin[:].rearrange("p (bb r pw) -> p bb r pw", bb=2, r=8)
            for jj in range(2):
                sxd = work.tile([128, 512], dt.bfloat16, name="sxd")
                sxdv = sxd[:].rearrange("p (r i pw) -> p r i pw", r=8, i=2)
                nc.vector.tensor_copy(sxdv[:, :, :, :], sxv[:, :, :, :, jj])
                for i in range(2):
                    m = 2 * i + jj
                    for j in range(4):
                        dsl = qinv[32 * m + j:32 * m + j + 29:4, b, :, :]
                        ssl = sxdv[32 * j:32 * j + 8, :, i, :]
                        nc.gpsimd.dma_start(dsl, ssl)

        qc = _quadform(nc, ps, work, big, ct, qin, 512, 'qWsq', 'qWB', 'qWC', 128, "qc")

        # pool stage input [128 = 4m' x (4c+w), 512 = (b, ph2:16, pw2:16)]
        pin = big.tile([128, 512], dt.float32, name="pin")
        qcv = qc[:].rearrange("p (bb ph pw2 two) -> p bb ph pw2 two", bb=2, ph=8, two=2)
        pinv = pin[:].rearrange("p (bb ph2 pw2) -> p bb ph2 pw2", bb=2, ph2=16)
        for jj in range(2):
            qcd = work.tile([128, 256], dt.float32, name="qcd")
            qcdv = qcd[:].rearrange("p (bb ph pw2) -> p bb ph pw2", bb=2, ph=8)
            nc.vector.tensor_copy(qcdv[:, :, :, :], qcv[:, :, :, :, jj])
            for i in range(2):
                m = 2 * i + jj
                for w in range(4):
                    for p2l in range(2):
                        dsl = pinv[32 * m + w:32 * m + w + 29:4, :, p2l:16:2, :]
                        sp = 32 * w + 2 * p2l + i
                        ssl = qcdv[sp:sp + 29:4, :, :, :]
                        nc.gpsimd.dma_start(dsl, ssl)

        z0 = _quadform(nc, ps, work, big, ct, pin, 512, 'pWsq', 'pWB', 'pWC', 32, "z0")

        # attention input xat[32*pw2l + s', b*64 + ph2*4 + pw2h]
        xat = big.tile([128, 128], dt.float32, name="xat")
        z0v = z0[0:32, :].rearrange("p (bb ph2 pwh pwl) -> p bb ph2 pwh pwl", bb=2, ph2=16, pwh=4)
        xatv = xat[:].rearrange("p (bb ph2 pwh) -> p bb ph2 pwh", bb=2, ph2=16)
        for pwl in range(4):
            nc.vector.tensor_copy(xatv[32 * pwl:32 * pwl + 32, :, :, :], z0v[:, :, :, :, pwl])

        _attention_classifier(nc, ps, ps2, work, big, ct, xat, out_t)


def _conv1b_stripe(nc, ps, work, ct, s1, stripe, rt1b, s2a):
    cur = s1[stripe % 3]
    y0 = 64 * stripe
    rtv = rt1b.rearrange("p (rr c) -> p rr c", c=258)
    for d_ in range(3):
        for j in range(4):
            src = cur[:].rearrange("p (r par x) -> p r par x", r=8, par=2)
            for par in range(2):
                rho0 = 2 * j + par - (d_ - 1)
                r_lo = 0 if rho0 >= 0 else 1
                r_hi = 7 if rho0 + 56 >= 64 else 8
                if r_lo >= r_hi:
                    continue
                dsl = rtv[16 * d_:16 * d_ + 16,
                          rho0 + 8 * r_lo: rho0 + 8 * (r_hi - 1) + 1: 8, 1:257]
                ssl = src[32 * j:32 * j + 16, r_lo:r_hi, par, :]
                nc.gpsimd.dma_start(dsl, ssl)
    if stripe < 3:
        nxt = s1[(stripe + 1) % 3]
        srcn = nxt[:].rearrange("p (r par x) -> p r par x", r=8, par=2)
        nc.gpsimd.dma_start(rtv[32:48, 63:64, 1:257], srcn[0:16, 0:1, 0, :])
    else:
        nc.vector.memset(rtv[32:48, 63, 1:257], 0.0)
    if stripe > 0:
        prev = s1[(stripe - 1) % 3]
        srcp = prev[:].rearrange("p (r par x) -> p r par x", r=8, par=2)
        nc.gpsimd.dma_start(rtv[0:16, 0:1, 1:257], srcp[96:112, 7:8, 1, :])
    else:
        nc.vector.memset(rtv[0:16, 0, 1:257], 0.0)

    W = ct['w1b']
    for r in range(8):
        acc = ps.tile([128, 512], dt.float32, name="ps1b", tag="pp")
        for j in range(4):
            y = 8 * r + 2 * j
            for dx in range(3):
                rhs = rtv[0:48, y:y + 2, dx:dx + 256]
                nc.tensor.matmul(acc[32 * j:32 * j + 32, :], W[:, 32 * dx:32 * dx + 32], rhs,
                                 start=(dx == 0), stop=(dx == 2), tile_position=(0, 32 * j))
        pre = work.tile([128, 512], dt.float32, name="pre1b")
        nc.scalar.activation(pre[:], acc[:], AF.Identity, bias=0.0, scale=1.0)
        m1 = work.tile([128, 256], dt.float32, name="m1b")
        nc.vector.tensor_tensor(m1[:], pre[:, 0:256], pre[:, 256:512], ALU.max)
        m1v = m1[:].rearrange("p (k two) -> p k two", two=2)
        m2 = work.tile([128, 128], dt.float32, name="m2b")
        nc.vector.tensor_tensor(m2[:], m1v[:, :, 0], m1v[:, :, 1], ALU.max)
        nc.scalar.activation(s2a[:, 128 * (8 * stripe + r):128 * (8 * stripe + r + 1)],
                             m2[:], AF.Relu, bias=ct['b1b'][:], scale=1.0)


def _conv_mid(nc, ps, ct, rt2a, s2b, wname, bname, K, pitch):
    W = ct[wname]
    rtv = rt2a.rearrange("p (rr c) -> p rr c", c=pitch)
    for r in range(8):
        acc = ps.tile([128, 512], dt.float32, name="ps2a", tag="pp")
        for j in range(4):
            y = 16 * r + 4 * j
            for dx in range(3):
                rhs = rtv[0:K, y:y + 4, dx:dx + 128]
                nc.tensor.matmul(acc[32 * j:32 * j + 32, :], W[:, 32 * dx:32 * dx + 32], rhs,
                                 start=(dx == 0), stop=(dx == 2), tile_position=(0, 32 * j))
        nc.scalar.activation(s2b[:, 512 * r:512 * (r + 1)], acc[:],
                             AF.Relu, bias=ct[bname][:], scale=1.0)


def _conv2b(nc, ps, work, ct, rt2b, sx_b):
    W = ct['w2b']
    rtv = rt2b.rearrange("p (rr c) -> p rr c", c=130)
    for r in range(8):
        acc = ps.tile([128, 512], dt.float32, name="ps2b", tag="pp")
        for j in range(4):
            y = 16 * r + 4 * j
            for dx in range(3):
                rhs = rtv[0:96, y:y + 4, dx:dx + 128]
                nc.tensor.matmul(acc[32 * j:32 * j + 32, :], W[:, 32 * dx:32 * dx + 32], rhs,
                                 start=(dx == 0), stop=(dx == 2), tile_position=(0, 32 * j))
        pre = work.tile([128, 512], dt.float32, name="pre2b")
        nc.scalar.activation(pre[:], acc[:], AF.Identity, bias=0.0, scale=1.0)
        accv = pre[:].rearrange("p (s x) -> p s x", s=4)
        m1 = work.tile([128, 256], dt.float32, name="m12b")
        m1v = m1[:].rearrange("p (s x) -> p s x", s=2)
        nc.vector.tensor_tensor(m1v[:, :, :], accv[:, 0:4:2, :], accv[:, 1:4:2, :], ALU.max)
        m1p = m1[:].rearrange("p (s x two) -> p s x two", s=2, two=2)
        m2 = work.tile([128, 128], dt.float32, name="m22b")
        m2v = m2[:].rearrange("p (s x) -> p s x", s=2)
        nc.vector.tensor_tensor(m2v[:, :, :], m1p[:, :, :, 0], m1p[:, :, :, 1], ALU.max)
        nc.scalar.activation(sx_b[:, 128 * r:128 * (r + 1)], m2[:],
                             AF.Relu, bias=ct['b2b'][:], scale=1.0)


def _quadform(nc, ps, work, big, ct, qin, N, wsq, wb, wc, M, tag):
    sq = work.tile([128, N], dt.float32, name="sq_q")
    nc.vector.tensor_mul(sq[:], qin[:], qin[:])
    # shifted copies so every tensor_tensor has all operands at one start partition
    shD = work.tile([128, N], dt.float32, name="shD_q")
    for k in range(3):
        nc.vector.tensor_copy(shD[32 * k:32 * k + 32, :], qin[32 * k + 32:32 * k + 64, :])
    nc.vector.tensor_copy(shD[96:128, :], qin[0:32, :])
    shC = work.tile([64, N], dt.float32, name="shC_q")
    nc.vector.tensor_copy(shC[0:32, :], qin[64:96, :])
    nc.vector.tensor_copy(shC[32:64, :], qin[96:128, :])
    prB = work.tile([128, N], dt.float32, name="prB_q")
    nc.vector.tensor_mul(prB[0:96, :], qin[0:96, :], shD[0:96, :])
    nc.vector.tensor_mul(prB[96:128, :], qin[96:128, :], shD[96:128, :])
    prC = work.tile([64, N], dt.float32, name="prC_q")
    nc.vector.tensor_mul(prC[0:64, :], qin[0:64, :], shC[0:64, :])
    acc = ps.tile([128, N], dt.float32, name="psq_q", tag="pp")
    nc.tensor.matmul(acc[0:M, :], ct[wsq][:, 0:M], sq[:], start=True, stop=False)
    nc.tensor.matmul(acc[0:M, :], ct[wb][:, 0:M], prB[:], start=False, stop=False)
    nc.tensor.matmul(acc[0:M, :], ct[wc][0:64, 0:M], prC[:], start=False, stop=True)
    n2ps = ps.tile([32, N], dt.float32, name="psn2_q", tag="pp")
    nc.tensor.matmul(n2ps[:], ct['qWn2'][:, 0:32], sq[:], start=True, stop=True)
    nt = work.tile([32, N], dt.float32, name="nt_q")
    nc.scalar.activation(nt[:], n2ps[:], AF.Sqrt)
    eps = work.tile([32, 1], dt.float32, name="eps_q")
    nc.vector.memset(eps[:], 1e-8)
    nte = work.tile([32, N], dt.float32, name="nte_q")
    nc.vector.tensor_scalar_add(nte[:], nt[:], eps[:])
    nsq = work.tile([32, N], dt.float32, name="nsq_q")
    nc.vector.tensor_mul(nsq[:], nte[:], nte[:])
    u2 = big.tile([128, N], dt.float32, name="u2_q")
    nc.vector.reciprocal(u2[0:32, :], nsq[:])
    if M > 32:
        nc.vector.tensor_copy(u2[32:64, :], u2[0:32, :])
        nc.vector.tensor_copy(u2[64:96, :], u2[0:32, :])
        nc.vector.tensor_copy(u2[96:128, :], u2[0:32, :])
    res = big.tile([128, N], dt.float32, name="res_q")
    nc.vector.tensor_mul(res[0:M, :], acc[0:M, :], u2[0:M, :])
    return res


def _attention_classifier(nc, ps, ps2, work, big, ct, xat, out_t):
    qkv = {}
    for nm, wn, bn_ in (("q", 'wqT', 'bq'), ("k", 'wkT', 'bk'), ("v", 'wvT', 'bv')):
        acc = ps.tile([128, 128], dt.float32, name=f"psqkv_{nm}", tag="pp")
        nc.tensor.matmul(acc[:], ct[wn][:], xat[:], start=True, stop=True)
        t = big.tile([128, 128], dt.float32, name=f"t_{nm}")
        nc.scalar.activation(t[:], acc[:], AF.Identity, bias=ct[bn_][:], scale=1.0)
        qkv[nm] = t
    Q, K, V = qkv["q"], qkv["k"], qkv["v"]

    # scores: stationary = blockdiag(Q-head-pair) [32, 128=(hh,q)], moving = K slice → P[(hh,q), k]
    P = big.tile([128, 64 * 8], dt.float32, name="Pmat")
    for b in range(2):
        for hp in range(4):
            bd = work.tile([128, 128], dt.float32, name="bdq")
            nc.vector.memset(bd[0:32, :], 0.0)
            nc.vector.tensor_copy(bd[0:16, 0:64], Q[32 * hp:32 * hp + 16, 64 * b:64 * b + 64])
            nc.gpsimd.dma_start(bd[16:32, 64:128], Q[32 * hp + 16:32 * hp + 32, 64 * b:64 * b + 64])
            ks = work.tile([128, 64], dt.float32, name="kslice")
            nc.vector.tensor_copy(ks[0:32, :], K[32 * hp:32 * hp + 32, 64 * b:64 * b + 64])
            sc = ps2.tile([128, 64], dt.float32, name="pssc", tag="pq")
            nc.tensor.matmul(sc[:], bd[0:32, :], ks[0:32, :], start=True, stop=True)
            rmax = work.tile([128, 1], dt.float32, name="rmax")
            nc.vector.reduce_max(rmax[:], sc[:], axis=mybir.AxisListType.X)
            nmx = work.tile([128, 1], dt.float32, name="nmx")
            nc.vector.tensor_scalar_mul(nmx[:], rmax[:], -0.25)
            e = work.tile([128, 64], dt.float32, name="esc")
            nc.scalar.activation(e[:], sc[:], AF.Exp, bias=nmx[:], scale=0.25)
            ssum = work.tile([128, 1], dt.float32, name="ssum")
            nc.vector.reduce_sum(ssum[:], e[:], axis=mybir.AxisListType.X)
            rinv = work.tile([128, 1], dt.float32, name="rinv")
            nc.vector.reciprocal(rinv[:], ssum[:])
            nc.vector.tensor_scalar_mul(P[:, (4 * b + hp) * 64:(4 * b + hp) * 64 + 64],
                                        e[:], rinv[:])

    PT = big.tile([64, 64 * 16], dt.float32, name="PT")
    for b in range(2):
        for hp in range(4):
            for hh in range(2):
                h = 2 * hp + hh
                ptmp = work.tile([128, 64], dt.float32, name="ptmp")
                for k in range(2):
                    nc.vector.tensor_copy(
                        ptmp[32 * k:32 * k + 32, :],
                        P[64 * hh + 32 * k:64 * hh + 32 * k + 32,
                          (4 * b + hp) * 64:(4 * b + hp) * 64 + 64])
                tp = ps2.tile([64, 64], dt.float32, name="pstp", tag="pq")
                nc.tensor.transpose(tp[:], ptmp[0:64, :], ct['ident'][0:64, 0:64])
                nc.vector.tensor_copy(PT[:, (8 * b + h) * 64:(8 * b + h) * 64 + 64], tp[:])
    tv = ps2.tile([128, 128], dt.float32, name="pstv", tag="pq")
    nc.tensor.transpose(tv[:], V[:], ct['ident'][:])
    VT = [big.tile([64, 128], dt.float32, name=f"VT_{b}") for b in range(2)]
    for b in range(2):
        for k in range(2):
            nc.vector.tensor_copy(VT[b][32 * k:32 * k + 32, :],
                                  tv[64 * b + 32 * k:64 * b + 32 * k + 32, :])

    Ops = ps.tile([64, 256], dt.float32, name="psO", tag="pp")
    for b in range(2):
        for h in range(8):
            nc.tensor.matmul(Ops[:, 128 * b + 16 * h:128 * b + 16 * h + 16],
                             PT[:, (8 * b + h) * 64:(8 * b + h) * 64 + 64],
                             VT[b][:, 16 * h:16 * h + 16],
                             start=True, stop=True)
    Osb = big.tile([64, 256], dt.float32, name="Osb")
    nc.vector.tensor_copy(Osb[:], Ops[:])

    mps = ps2.tile([128, 2], dt.float32, name="psmean", tag="pq")
    for b in range(2):
        nc.tensor.matmul(mps[:, b:b + 1], Osb[:, 128 * b:128 * b + 128],
                         ct['ones64'][:], start=True, stop=True)
    matt = big.tile([128, 2], dt.float32, name="matt")
    nc.vector.tensor_copy(matt[:], mps[:])

    accp = ps.tile([128, 2], dt.float32, name="psproj", tag="pp")
    nc.tensor.matmul(accp[:], ct['woT'][:], matt[:], start=True, stop=True)
    att = big.tile([128, 2], dt.float32, name="attf")
    nc.scalar.activation(att[:], accp[:], AF.Identity, bias=ct['bo'][:], scale=1.0)

    h1a_ps = ps.tile([128, 2], dt.float32, name="psh1a", tag="pp")
    nc.tensor.matmul(h1a_ps[:], ct['fc1aT'][:], att[:], start=True, stop=True)
    h1b_ps = ps.tile([128, 2], dt.float32, name="psh1b", tag="pp")
    nc.tensor.matmul(h1b_ps[:], ct['fc1bT'][:], att[:], start=True, stop=True)
    h1a = big.tile([128, 2], dt.float32, name="h1a")
    nc.scalar.activation(h1a[:], h1a_ps[:], AF.Relu, bias=ct['bfc1a'][:], scale=1.0)
    h1b = big.tile([128, 2], dt.float32, name="h1b")
    nc.scalar.activation(h1b[:], h1b_ps[:], AF.Relu, bias=ct['bfc1b'][:], scale=1.0)
    h2ps = ps.tile([128, 2], dt.float32, name="psh2", tag="pp")
    nc.tensor.matmul(h2ps[:], ct['fc2aT'][:], h1a[:], start=True, stop=False)
    nc.tensor.matmul(h2ps[:], ct['fc2bT'][:], h1b[:], start=False, stop=True)
    h2 = big.tile([128, 2], dt.float32, name="h2t")
    nc.scalar.activation(h2[:], h2ps[:], AF.Relu, bias=ct['bfc2'][:], scale=1.0)
    h3ps = ps2.tile([32, 2], dt.float32, name="psh3", tag="pq")
    nc.tensor.matmul(h3ps[:], ct['fc3T'][:], h2[:], start=True, stop=True)
    logits = big.tile([32, 2], dt.float32, name="logits")
    nc.vector.tensor_scalar_add(logits[:], h3ps[:], ct['bfc3'][:])
    for b in range(2):
        nc.sync.dma_start(out_t[b].rearrange("c -> () c"), logits[0:10, b:b + 1])


def kernel(x, params):
    global _PROGRAM
    x = np.asarray(x, np.float32)
    if _PROGRAM is None:
        _PROGRAM = build_program()
    nc = _PROGRAM
    consts = make_consts(params)
    in_maps = []
    for c in range(N_CORES):
        m = {"x": np.ascontiguousarray(x[B_PER * c:B_PER * (c + 1), 0])}
        for name, shape, d in CONST_SPECS:
            v = consts[name]
            if d == dt.bfloat16:
                v = v.astype(BF)
            m[name] = np.ascontiguousarray(v)
        in_maps.append(m)
    res = run_bass_kernel_spmd(nc, in_maps, list(range(N_CORES)))
    return np.concatenate([res.results[c]["out"] for c in range(N_CORES)], axis=0)
